# revision 4
# baseline (speedup 1.0000x reference)
"""Bahdanau attention Trainium2 kernel.

Full-input contract: kernel(**inputs) takes the unsharded inputs
(query [32,1,1024], keys [32,2048,1024], Wa_w/Wa_b/Ua_w/Ua_b/Va_w/Va_b)
and returns (context [32,1,1024], weights [32,1,2048]) as float32,
matching reference().

Strategy: data-parallel over batch across 8 NeuronCores (4 batches per
core). Host pre-transposes and casts keys to bf16 [b, h, s] so the
dominant matmul k_proj^T = Ua^T @ keys^T streams directly from DRAM with
no on-device transposes. Per core:
  stage 1 (PE):  e^T[h,s] = tanh(Ua^T keys^T + (q W_a + Wa_b + Ua_b))
                 bf16 matmuls, fp32 PSUM accum, tanh+bias fused on ACT
  stage 2 (PE):  scores[1,s] = Va^T e^T accumulated over h chunks
  softmax (DVE/ACT): on scores row, fp32
  stage 3 (DVE): context^T[h,1] = sum_s keysT[h,s] * w[s] via
                 tensor_tensor_reduce against DMA-broadcast weights
Va_b is dropped: softmax is shift-invariant so it cancels exactly.
"""

import os

os.environ.setdefault("JAX_PLATFORMS", "axon")

import numpy as np
import ml_dtypes
from contextlib import ExitStack

import concourse.bass as bass
import concourse.bacc as bacc
import concourse.tile as tile
from concourse import mybir

B, S, H = 32, 2048, 1024
NCORES = 8
BL = B // NCORES  # batches per core
P = 128
HC = H // P  # h chunks
ST = 512  # s tile for stage 1/2
NST = S // ST
F32 = mybir.dt.float32
BF16 = mybir.dt.bfloat16
NPBF16 = ml_dtypes.bfloat16


def build_core_program(tc, outs, ins):
    """Emit the per-core Tile program.

    outs: dict with APs ctx_out [BL, H] f32, w_out [BL, S] f32
    ins:  dict with APs keysT [BL, H, S] bf16, ua [H, H] bf16,
          wa [H, H] bf16, va [H, 1] bf16, qT [H, BL] bf16,
          bcomb [H] f32 (Wa_b + Ua_b)
    """
    nc = tc.nc
    Tanh = mybir.ActivationFunctionType.Tanh
    Exp = mybir.ActivationFunctionType.Exp
    mult = mybir.AluOpType.mult
    add = mybir.AluOpType.add
    amax = mybir.AluOpType.max

    with ExitStack() as ctx:
        consts = ctx.enter_context(tc.tile_pool(name="consts", bufs=1))
        ktp = ctx.enter_context(tc.tile_pool(name="ktp", bufs=2))
        epool = ctx.enter_context(tc.tile_pool(name="epool", bufs=3))
        wbcp = ctx.enter_context(tc.tile_pool(name="wbcp", bufs=2))
        smx = ctx.enter_context(tc.tile_pool(name="smx", bufs=2))
        dramp = ctx.enter_context(tc.tile_pool(name="dramp", bufs=2, space="DRAM"))
        psum_e = ctx.enter_context(tc.tile_pool(name="psum_e", bufs=3, space="PSUM"))
        psum_s = ctx.enter_context(tc.tile_pool(name="psum_s", bufs=2, space="PSUM"))
        psum_q = ctx.enter_context(tc.tile_pool(name="psum_q", bufs=2, space="PSUM"))

        # ---- constants: weights in chunked [p, chunk, ...] layout
        ua_sb = consts.tile([P, HC, H], BF16)
        nc.sync.dma_start(out=ua_sb, in_=ins["ua"].rearrange("(c p) j -> p c j", p=P))
        wa_sb = consts.tile([P, HC, H], BF16)
        nc.sync.dma_start(out=wa_sb, in_=ins["wa"].rearrange("(c p) j -> p c j", p=P))
        va_sb = consts.tile([P, HC, 1], BF16)
        nc.sync.dma_start(out=va_sb, in_=ins["va"].rearrange("(c p) o -> p c o", p=P))
        qt_sb = consts.tile([P, HC, BL], BF16)
        nc.sync.dma_start(out=qt_sb, in_=ins["qT"].rearrange("(c p) b -> p c b", p=P))
        bc_sb = consts.tile([P, HC], F32)
        nc.sync.dma_start(out=bc_sb, in_=ins["bcomb"].rearrange("(c p) -> p c", p=P))

        # ---- q_proj for all local batches; bias[p, co, b] = qW[co*P+p, b] + bcomb
        bias_sb = consts.tile([P, HC, BL], F32)
        for co in range(HC):
            pq = psum_q.tile([P, BL], F32)
            for ci in range(HC):
                nc.tensor.matmul(
                    pq,
                    lhsT=wa_sb[:, ci, co * P : (co + 1) * P],
                    rhs=qt_sb[:, ci, :],
                    start=(ci == 0),
                    stop=(ci == HC - 1),
                )
            nc.vector.tensor_scalar_add(bias_sb[:, co, :], pq, bc_sb[:, co : co + 1])

        keysT = ins["keysT"].rearrange("b (c p) s -> p b c s", p=P)

        for b in range(BL):
            kt = ktp.tile([P, HC, S], BF16)
            for st in range(NST):
                sl = slice(st * ST, (st + 1) * ST)
                nc.sync.dma_start(out=kt[:, :, sl], in_=keysT[:, b, :, sl])

            # ---- stage 1+2: e^T = tanh(Ua^T K^T + bias); scores = Va^T e^T
            sc_sb = smx.tile([1, S], F32)
            for st in range(NST):
                sl = slice(st * ST, (st + 1) * ST)
                ps = psum_s.tile([1, ST], F32)
                for co in range(HC):
                    pe = psum_e.tile([P, ST], F32)
                    for ci in range(HC):
                        nc.tensor.matmul(
                            pe,
                            lhsT=ua_sb[:, ci, co * P : (co + 1) * P],
                            rhs=kt[:, ci, sl],
                            start=(ci == 0),
                            stop=(ci == HC - 1),
                        )
                    e_sb = epool.tile([P, ST], BF16)
                    nc.scalar.activation(
                        out=e_sb, in_=pe, func=Tanh, bias=bias_sb[:, co, b : b + 1]
                    )
                    nc.tensor.matmul(
                        ps,
                        lhsT=va_sb[:, co, :],
                        rhs=e_sb,
                        start=(co == 0),
                        stop=(co == HC - 1),
                    )
                nc.vector.tensor_copy(out=sc_sb[:, sl], in_=ps)

            # ---- softmax on the [1, S] scores row (fp32)
            mx = smx.tile([1, 1], F32)
            nc.vector.tensor_reduce(mx, sc_sb, axis=mybir.AxisListType.X, op=amax)
            neg_mx = smx.tile([1, 1], F32)
            nc.scalar.mul(neg_mx, mx, -1.0)
            wexp = smx.tile([1, S], F32)
            ssum = smx.tile([1, 1], F32)
            nc.scalar.activation(
                out=wexp, in_=sc_sb, func=Exp, bias=neg_mx, accum_out=ssum
            )
            rsum = smx.tile([1, 1], F32)
            nc.vector.reciprocal(rsum, ssum)
            wrow = smx.tile([1, S], F32)
            nc.vector.tensor_scalar_mul(wrow, wexp, rsum)
            nc.sync.dma_start(out=outs["w_out"][b : b + 1, :], in_=wrow)

            # weights row -> DRAM bounce -> broadcast to 128 partitions (fp32)
            wd = dramp.tile([1, S], F32)
            nc.sync.dma_start(out=wd, in_=wrow)
            wbc = wbcp.tile([P, S], F32)
            nc.sync.dma_start(out=wbc, in_=wd.broadcast_to([P, S]))

            # ---- stage 3: context^T[h] = sum_s keysT[h, s] * w[s]
            # (tensor_tensor_reduce faults on this HW path; use mul+reduce)
            ctxt = smx.tile([P, HC], F32)
            for c in range(HC):
                prod = wbcp.tile([P, S], F32)
                nc.vector.tensor_mul(prod, kt[:, c, :], wbc)
                nc.vector.tensor_reduce(
                    ctxt[:, c : c + 1], prod, axis=mybir.AxisListType.X, op=add
                )
            nc.sync.dma_start(
                out=outs["ctx_out"][b].rearrange("(c p) -> p c", p=P), in_=ctxt
            )


def build_nc():
    nc = bacc.Bacc("TRN2", target_bir_lowering=False, debug=False)
    ins = {
        "keysT": nc.dram_tensor("keysT", [BL, H, S], BF16, kind="ExternalInput"),
        "ua": nc.dram_tensor("ua", [H, H], BF16, kind="ExternalInput"),
        "wa": nc.dram_tensor("wa", [H, H], BF16, kind="ExternalInput"),
        "va": nc.dram_tensor("va", [H, 1], BF16, kind="ExternalInput"),
        "qT": nc.dram_tensor("qT", [H, BL], BF16, kind="ExternalInput"),
        "bcomb": nc.dram_tensor("bcomb", [H], F32, kind="ExternalInput"),
    }
    outs = {
        "ctx_out": nc.dram_tensor("ctx_out", [BL, H], F32, kind="ExternalOutput"),
        "w_out": nc.dram_tensor("w_out", [BL, S], F32, kind="ExternalOutput"),
    }
    with tile.TileContext(nc) as tc:
        build_core_program(
            tc,
            {k: v.ap() for k, v in outs.items()},
            {k: v.ap() for k, v in ins.items()},
        )
    nc.compile()
    return nc


_NC_CACHE = []


def prepare_in_maps(query, keys, Wa_w, Wa_b, Ua_w, Ua_b, Va_w, Va_b):
    query = np.asarray(query, dtype=np.float32)
    keys = np.asarray(keys, dtype=np.float32)
    ua_h = np.asarray(Ua_w, dtype=np.float32).astype(NPBF16)
    wa_h = np.asarray(Wa_w, dtype=np.float32).astype(NPBF16)
    va_h = np.asarray(Va_w, dtype=np.float32).astype(NPBF16)
    bcomb = (np.asarray(Wa_b, np.float32) + np.asarray(Ua_b, np.float32)).astype(
        np.float32
    )
    in_maps = []
    for c in range(NCORES):
        bs = slice(c * BL, (c + 1) * BL)
        in_maps.append(
            {
                "keysT": keys[bs].transpose(0, 2, 1).astype(NPBF16, order="C"),
                "ua": ua_h,
                "wa": wa_h,
                "va": va_h,
                "qT": query[bs, 0, :].T.astype(NPBF16, order="C"),
                "bcomb": bcomb,
            }
        )
    return in_maps


def kernel(query, keys, Wa_w, Wa_b, Ua_w, Ua_b, Va_w, Va_b):
    from concourse.bass_utils import run_bass_kernel_spmd

    if not _NC_CACHE:
        _NC_CACHE.append(build_nc())
    nc = _NC_CACHE[0]
    in_maps = prepare_in_maps(query, keys, Wa_w, Wa_b, Ua_w, Ua_b, Va_w, Va_b)
    res = run_bass_kernel_spmd(nc, in_maps, list(range(NCORES)))
    ctx = np.concatenate([np.asarray(r["ctx_out"]) for r in res.results], axis=0)
    wts = np.concatenate([np.asarray(r["w_out"]) for r in res.results], axis=0)
    return (
        ctx.reshape(B, 1, H).astype(np.float32),
        wts.reshape(B, 1, S).astype(np.float32),
    )
